# revision 15
# baseline (speedup 1.0000x reference)
"""Dilated segment attention on 8 TRN2 NeuronCores (Bass/Tile).

Problem (hardcoded from spec):
  x [2, 8192, 2048] f32, Wqkv [6144, 2048], b_qkv [6144], Wout [2048, 2048],
  b_out [2048].  segment=512, dilation=2 -> 16 segments of L=256 dilated
  tokens per segment per batch; per-segment 16-head attention (hd=128);
  fused qkv and out projections.  Output [2, 4096, 2048] f32.

Sharding: the 32 (batch, segment) instances are independent -> 4 per core.
Host pre-gathers the dilated tokens, pre-transposes/pre-tiles operands and
casts to bf16 (compute precision; measured end-to-end rel err ~5e-3).

The kernel is PE-bound (93%+ occupancy): 1.11M matmul columns/core at
2.4GHz is ~464us.  Beyond the baseline, this version
  - computes softmax denominators with a per-head batched GpSimd
    partition_all_reduce instead of ones-matmuls (-32k PE columns),
  - interleaves attention seg-pairs between the q- and k-projection
    chunks so the ACT exp latency always hides under ~7us of projection,
  - emits the output projection feature-major (outT[e, tok]) so the
    stationary operand is a Wout tile reused for 1024 moving columns
    (half the LDWEIGHTS of the token-major form); host transposes back,
  - streams xst by token-quarters and runs the first v-chunk
    quarter-major so the PE starts ~4us earlier during the cold 4MB
    xst delivery.

Per-core dataflow (all matmuls K=128, bf16):
  qkv proj   : feature-major  qkvT[e, tok] = W-tile.T @ xsT-tile  (accum 16)
  scores     : scoresT[lk, lq] = kT.T @ qT  (per seg, head)
  softmax    : exp on ScalarE (scale=1/sqrt(hd); scores provably in
               [-6, 6] so no max subtraction); denominators via DVE
               chunk-add + GpSimd partition_all_reduce; normalize on DVE
               at the AV psum drain.
  AV         : outT[hd, lq] = v[lk, hd].T @ expT[lk, lq]
  out proj   : outT[e, tok] = WoutT-tile.T @ aT-tile  (accum 16 heads)
b_out is applied on the host (purely linear post-op); b_qkv is applied
on-chip (ScalarE bias) since it feeds the softmax nonlinearity.
"""

import numpy as np
import ml_dtypes

B = 2
S = 8192
D = 2048
H = 16
HD = 128
SEGMENT = 512
DIL = 2
NSEG = S // SEGMENT          # 16
L = SEGMENT // DIL           # 256 dilated tokens per segment
N_CORES = 8
PAIRS = B * NSEG             # 32 independent (b, n) instances
SPC = PAIRS // N_CORES       # 4 segments per core
TOK = SPC * L                # 1024 tokens per core
DT = D // 128                # 16 contraction tiles
NCHUNK = 3 * D // 128        # 48 qkv feature chunks (16 q, 16 k, 16 v)
SCALE = 1.0 / float(np.sqrt(HD))

_PROGRAM = None


def _build_program():
    import concourse.bass as bass
    import concourse.bacc as bacc
    import concourse.tile as tile
    from concourse import mybir
    from concourse import bass_isa

    BF = mybir.dt.bfloat16
    F32 = mybir.dt.float32
    ts = bass.ts

    nc = bacc.Bacc("TRN2", target_bir_lowering=False, debug=False,
                   num_devices=N_CORES)

    xst_d = nc.dram_tensor("xst", [4, 128, DT, 256], BF, kind="ExternalInput")
    wqkv_d = nc.dram_tensor("wqkv_t", [NCHUNK, 128, DT * 128], BF,
                            kind="ExternalInput")
    wout_d = nc.dram_tensor("wout_t", [DT, 128, H * 128], BF,
                            kind="ExternalInput")
    bq_d = nc.dram_tensor("bq_t", [128, NCHUNK], F32, kind="ExternalInput")
    out_d = nc.dram_tensor("out", [D, TOK], F32, kind="ExternalOutput")

    with tile.TileContext(nc) as tc:
        with (
            tc.tile_pool(name="const", bufs=1) as const_p,
            tc.tile_pool(name="big", bufs=1) as big_p,
            tc.tile_pool(name="wq", bufs=8) as w_p,
            tc.tile_pool(name="qk", bufs=4) as qk_p,
            tc.tile_pool(name="vt", bufs=2) as vt_p,
            tc.tile_pool(name="ex", bufs=4) as ex_p,
            tc.tile_pool(name="st", bufs=2) as st_p,
            tc.tile_pool(name="ou", bufs=2) as ou_p,
            tc.tile_pool(name="pp", bufs=4, space="PSUM") as pp_p,
            tc.tile_pool(name="pa", bufs=2, space="PSUM") as pa_p,
        ):
            bq_sb = const_p.tile([128, NCHUNK], F32)
            nc.sync.dma_start(out=bq_sb[:], in_=bq_d[:])

            # Cold-start ordering: the PE's first work (v chunk 0, index
            # 32) needs the first quarter of its W chunk and the first
            # token-quarter of xst.  Both live in token-quarter-major
            # layouts so every piece is one fully-linear DMA, and the
            # first matmul can start after ~1.1MB instead of ~4.5MB.
            first_w = w_p.tile([128, DT * 128], BF, tag="w", name="first_w")
            nc.sync.dma_start(out=first_w[:, 0:512], in_=wqkv_d[32][:, 0:512])
            xst_sb = big_p.tile([128, 4, DT, 256], BF)
            nc.sync.dma_start(out=xst_sb[:, 0], in_=xst_d[0])
            for kk in range(1, 4):
                nc.sync.dma_start(out=first_w[:, ts(kk, 512)],
                                  in_=wqkv_d[32][:, ts(kk, 512)])
            for q in range(1, 4):
                nc.sync.dma_start(out=xst_sb[:, q], in_=xst_d[q])
            vtok_sb = big_p.tile([128, H, SPC * 2, 128], BF)
            # head-major so the out-projection's [seg-pair, head] reads have
            # tight per-head dependency ranges (head hh's matmuls wait only
            # on head hh's normalize, not on later heads)
            aT_sb = big_p.tile([128, H, SPC, L], BF)

            def proj_chunk(c, out_tile, wck=None, quarter_major=False):
                """qkvT chunk c: out_tile[128, TOK] bf16 = W-chunk.T @ xsT + b."""
                if wck is None:
                    wck = w_p.tile([128, DT * 128], BF, tag="w")
                    nc.sync.dma_start(out=wck[:], in_=wqkv_d[c])
                if quarter_major:
                    # first chunk while xst streams in: consume one
                    # token-quarter at a time so matmuls start on quarter 0
                    for q in range(4):
                        psq = pp_p.tile([128, 512], F32, tag="pp", name="psq")
                        for dt in range(DT):
                            nc.tensor.matmul(
                                psq[:, 0:256],
                                wck[:, ts(dt, 128)],
                                xst_sb[:, q, dt, :],
                                start=(dt == 0),
                                stop=(dt == DT - 1),
                            )
                        nc.scalar.activation(
                            out=out_tile[:, ts(q, 256)],
                            in_=psq[:, 0:256],
                            func=mybir.ActivationFunctionType.Identity,
                            bias=bq_sb[:, c:c + 1],
                            scale=1.0,
                        )
                    return
                pss = [pp_p.tile([128, 512], F32, tag="pp", name=f"ps{half}")
                       for half in range(2)]
                for dt in range(DT):
                    for half in range(2):
                        nc.tensor.matmul(
                            pss[half][:],
                            wck[:, ts(dt, 128)],
                            xst_sb[:, 2 * half:2 * half + 2, dt, :],
                            start=(dt == 0),
                            stop=(dt == DT - 1),
                        )
                for half in range(2):
                    nc.scalar.activation(
                        out=out_tile[:, ts(half, 512)],
                        in_=pss[half][:],
                        func=mybir.ActivationFunctionType.Identity,
                        bias=bq_sb[:, c:c + 1],
                        scale=1.0,
                    )

            # ---- v projection (feature-major) + transpose to token-major ----
            # One transposing DMA per head (xbar transpose, ~261GB/s): row
            # tok = tc*128+p of vt.T lands at vtok[p, tc, :], exactly the AV
            # stationary layout.  Emitted one chunk behind the projection so
            # the DMA never waits on the ScalarE psum->sbuf drain.
            def v_transposes(h, vt_tile):
                nc.sync.dma_start(out=vtok_sb[:, h, :, :], in_=vt_tile[:],
                                  transpose=True)

            prev_v = None
            for h in range(H):
                vt_tile = vt_p.tile([128, TOK], BF, tag="vt")
                proj_chunk(32 + h, vt_tile,
                           wck=first_w if h == 0 else None,
                           quarter_major=(h == 0))
                if prev_v is not None:
                    v_transposes(h - 1, prev_v)
                prev_v = vt_tile
            v_transposes(H - 1, prev_v)

            # ---- per-head attention, seg-pair interleaved with projection ----
            # scoresT[lk, lq] directly (operands swapped): exp is
            # layout-agnostic (scores provably small -> no max pass), expT
            # feeds AV untransposed.  Softmax denominators: DVE adds the two
            # lk chunks of expT, one batched GpSimd partition_all_reduce per
            # head sums over partitions (all partitions get the result), DVE
            # reciprocal + multiply normalize at the AV psum drain.
            # Schedule: the ~0.7us ACT exp of a seg-pair always has a full
            # projection chunk (~7us) between its scoresT and its AV, so the
            # PE stream never waits on ScalarE.
            head_state = {}

            def emit_scores_pair(h, qh, kh, pair):
                """scoresT + exp + chunk-add + denominator all-reduce for
                segs 2*pair, 2*pair+1.  The GpSimd all-reduce is emitted
                here (per pair, ~3.7us) so it runs during the next
                projection chunk, well before the pair's normalize."""
                st = head_state[h]
                es = ex_p.tile([128, 2, L], BF, tag="es", bufs=4, name="es")
                for i, seg in enumerate((2 * pair, 2 * pair + 1)):
                    scT = pa_p.tile([128, 2, L], F32, tag="pa", name="scT")
                    for lkc in range(2):
                        nc.tensor.matmul(
                            scT[:, lkc, :],
                            kh[:, seg * L + lkc * 128: seg * L + (lkc + 1) * 128],
                            qh[:, seg * L:(seg + 1) * L],
                        )
                    e_t = ex_p.tile([128, 2, L], BF, tag="ex")
                    nc.scalar.activation(
                        out=e_t[:],
                        in_=scT[:],
                        func=mybir.ActivationFunctionType.Exp,
                        scale=SCALE,
                    )
                    st["e"][seg] = e_t
                    nc.vector.tensor_add(es[:, i, :], e_t[:, 0, :], e_t[:, 1, :])
                den = st_p.tile([128, 2, L], F32, tag="den", bufs=2, name="den")
                nc.gpsimd.partition_all_reduce(
                    den[:], es[:], 128, bass_isa.ReduceOp.add)
                st["den"][pair] = den

            def emit_av_pair(h, pair):
                """AV for segs 2*pair, 2*pair+1 into one psum bank."""
                st = head_state[h]
                avs = pa_p.tile([128, 2, L], F32, tag="pav", bufs=2,
                                name="avs")
                st["av"][pair] = avs
                for i, seg in enumerate((2 * pair, 2 * pair + 1)):
                    e_t = st["e"][seg]
                    for lkc in range(2):
                        nc.tensor.matmul(
                            avs[:, i, :],
                            vtok_sb[:, h, seg * 2 + lkc, :],
                            e_t[:, lkc, :],
                            start=(lkc == 0),
                            stop=(lkc == 1),
                        )

            def emit_norm_pair(h, pair):
                """Normalize the pair's AV by its denominators -> aT_sb."""
                st = head_state[h]
                inv = st_p.tile([128, 2, L], F32, tag="inv", bufs=2, name="inv")
                nc.vector.reciprocal_approx_fast(out=inv[:], in_=st["den"][pair][:])
                avs = st["av"][pair]
                for i, seg in enumerate((2 * pair, 2 * pair + 1)):
                    nc.vector.tensor_mul(
                        aT_sb[:, h, seg, :], avs[:, i, :], inv[:, i, :])

            def start_head(h, qh, kh):
                head_state[h] = {
                    "q": qh, "k": kh,
                    "e": [None] * SPC,
                    "den": [None] * 2,
                    "av": [None] * 2,
                }

            # Prefetch the first two Wout e-chunk blocks now: their
            # dma_starts land early in the Sync stream, so the transfers
            # run during the attention phase instead of stalling the
            # out-projection start by ~3us.
            wo_pre = []
            for ec in range(2):
                wo_ec = w_p.tile([128, H, 128], BF, tag="wo", bufs=2,
                                 name="wo_ec")
                nc.sync.dma_start(out=wo_ec[:], in_=wout_d[ec])
                wo_pre.append(wo_ec)

            prev = None
            for h in range(H):
                qh = qk_p.tile([128, TOK], BF, tag="qk")
                kh = qk_p.tile([128, TOK], BF, tag="qk")
                proj_chunk(h, qh)
                if prev is not None:
                    # block X: AV + normalize (prev, pair 0), scoresT(prev,
                    # pair 1)
                    emit_av_pair(prev, 0)
                    emit_norm_pair(prev, 0)
                    emit_scores_pair(prev, head_state[prev]["q"],
                                     head_state[prev]["k"], 1)
                proj_chunk(16 + h, kh)
                if prev is not None:
                    # block Y: AV + normalize (prev, pair 1), then
                    # scoresT(h, pair 0)
                    emit_av_pair(prev, 1)
                    emit_norm_pair(prev, 1)
                    del head_state[prev]
                start_head(h, qh, kh)
                emit_scores_pair(h, qh, kh, 0)
                prev = h
            # drain last head
            emit_av_pair(prev, 0)
            emit_norm_pair(prev, 0)
            emit_scores_pair(prev, head_state[prev]["q"],
                             head_state[prev]["k"], 1)
            emit_av_pair(prev, 1)
            emit_norm_pair(prev, 1)
            del head_state[prev]

            # ---- output projection (feature-major: outT[e, tok]) ----
            # Stationary = Wout tile (one per (e-chunk, head), reused for
            # 1024 moving columns -> 256 LDWEIGHTS total).  Wout streams in
            # sixteen 512KB e-chunk blocks (linear DMAs, 2 prefetched).
            for ec in range(DT):
                if ec < 2:
                    wo_ec = wo_pre[ec]
                else:
                    wo_ec = w_p.tile([128, H, 128], BF, tag="wo", bufs=2,
                                     name="wo_ec")
                    nc.sync.dma_start(out=wo_ec[:], in_=wout_d[ec])
                pos = [pp_p.tile([128, 512], F32, tag="pp", name=f"po{sp}")
                       for sp in range(2)]
                for hh in range(H):
                    for sp in range(2):
                        nc.tensor.matmul(
                            pos[sp][:],
                            wo_ec[:, hh, :],
                            aT_sb[:, hh, 2 * sp:2 * sp + 2, :],
                            start=(hh == 0),
                            stop=(hh == H - 1),
                        )
                for sp in range(2):
                    if ec == DT - 1 and sp == 1:
                        # split the last drain so the final store DMA
                        # starts half a tile earlier
                        for qq in range(2):
                            ob = ou_p.tile([128, 256], F32, tag="ou2")
                            nc.vector.tensor_copy(
                                out=ob[:], in_=pos[sp][:, ts(qq, 256)])
                            nc.sync.dma_start(
                                out=out_d[ec * 128:(ec + 1) * 128,
                                          sp * 512 + qq * 256:
                                          sp * 512 + (qq + 1) * 256],
                                in_=ob[:],
                            )
                    else:
                        ob = ou_p.tile([128, 512], F32, tag="ou")
                        nc.vector.tensor_copy(out=ob[:], in_=pos[sp][:])
                        nc.sync.dma_start(
                            out=out_d[ec * 128:(ec + 1) * 128,
                                      ts(sp, 512)],
                            in_=ob[:],
                        )

    nc.compile()
    _dedupe_ldweights(nc)
    return nc


def _dedupe_ldweights(nc):
    """Drop InstLdweights whose weights are already resident in the PE array.

    tile_legalize emits one LDWEIGHTS per matmul; consecutive matmuls that
    share the stationary operand (projection token-halves, out-proj seg
    pairs) reload identical weights, costing ~97ns of PE pipe each.  Walk
    each block's PE stream tracking the loaded-weights key and delete
    reloads.  Only semaphore-free LDWEIGHTS are dropped, so the sync graph
    is untouched; EVENT_SEMAPHORE/DRAIN between pairs don't disturb the
    array, any other PE instruction conservatively invalidates the key.
    """
    from concourse import mybir

    PE = mybir.EngineType.PE
    dropped = 0
    for f in nc.m.functions:
        for blk in f.blocks:
            insts = blk.instructions
            loaded = None
            to_drop = []
            for idx, x in enumerate(insts):
                if getattr(x, "engine", None) != PE:
                    continue
                nm = type(x).__name__
                if nm == "InstLdweights":
                    si = x.sync_info
                    clean = si is None or (not si.on_wait and not si.on_update)
                    key = (str(x.ins[0]), str(x.is_transpose),
                           str(x.perf_mode), str(x.tile_position))
                    if clean and loaded == key:
                        to_drop.append(idx)
                    else:
                        loaded = key
                elif nm == "InstMatmult":
                    continue
                elif nm in ("InstEventSemaphore", "InstDrain"):
                    continue
                else:
                    loaded = None
            for idx in reversed(to_drop):
                del insts[idx]
            blk.instructions = insts
            dropped += len(to_drop)
    return dropped


def get_program():
    global _PROGRAM
    if _PROGRAM is None:
        _PROGRAM = _build_program()
    return _PROGRAM


def make_in_maps(x, Wqkv, b_qkv):
    """Host-side shard + layout prep (bf16 casts, transposes, tiling)."""
    bf16 = ml_dtypes.bfloat16
    x = np.asarray(x, dtype=np.float32)
    Wqkv = np.asarray(Wqkv, dtype=np.float32)
    b_qkv = np.asarray(b_qkv, dtype=np.float32)

    xs = x.reshape(B, NSEG, SEGMENT, D)[:, :, ::DIL, :]     # [2,16,256,2048]
    xs_flat = xs.reshape(PAIRS, L, D)

    # lhsT tiles packed partition-major: wt[c, p, dt*128+j] = WqkvT[dt*128+p,
    # c*128+j] so one chunk is a single linear per-partition DMA.
    wt = np.ascontiguousarray(
        Wqkv.reshape(NCHUNK, 128, DT, 128).transpose(0, 3, 2, 1)
        .reshape(NCHUNK, 128, DT * 128)
    ).astype(bf16)                                          # [48,128,2048]
    bqt = np.ascontiguousarray(b_qkv.reshape(NCHUNK, 128).T)  # [128,48] f32

    in_maps = []
    for i in range(N_CORES):
        tok = xs_flat[SPC * i:SPC * (i + 1)].reshape(TOK, D)
        # token-quarter-major: xst[q, p, dt, j] = xsT[dt*128+p, q*256+j]
        # so each quarter is one fully-linear DMA.
        xst = np.ascontiguousarray(
            tok.T.reshape(DT, 128, 4, 256).transpose(2, 1, 0, 3)).astype(bf16)
        in_maps.append({"xst": xst, "wqkv_t": wt, "bq_t": bqt})
    return in_maps


def make_wout_tiled(Wout):
    Wout = np.asarray(Wout, dtype=np.float32)
    # wout_t[ec, p, h*128+j] = Wout[ec*128+j, h*128+p]: per e-chunk block
    # of per-head lhsT tiles, one linear 512KB DMA each.
    return np.ascontiguousarray(
        Wout.reshape(DT, 128, H, 128).transpose(0, 3, 2, 1)
        .reshape(DT, 128, H * 128)).astype(ml_dtypes.bfloat16)


def kernel(x, Wqkv, b_qkv, Wout, b_out):
    from concourse import bass_utils

    nc = get_program()
    in_maps = make_in_maps(x, Wqkv, b_qkv)
    wot = make_wout_tiled(Wout)
    for m in in_maps:
        m["wout_t"] = wot

    res = bass_utils.run_bass_kernel_spmd(
        nc, in_maps, core_ids=list(range(N_CORES)))
    # out is feature-major [D, TOK] per core -> transpose back to [TOK, D]
    outs = [np.ascontiguousarray(res.results[i]["out"].T)
            for i in range(N_CORES)]
    full = np.concatenate(outs, axis=0) + np.asarray(b_out, dtype=np.float32)
    return np.ascontiguousarray(full.reshape(B, NSEG * L, D), dtype=np.float32)


# revision 18
# speedup vs baseline: 1.0143x; 1.0143x over previous
"""Dilated segment attention on 8 TRN2 NeuronCores (Bass/Tile).

Problem (hardcoded from spec):
  x [2, 8192, 2048] f32, Wqkv [6144, 2048], b_qkv [6144], Wout [2048, 2048],
  b_out [2048].  segment=512, dilation=2 -> 16 segments of L=256 dilated
  tokens per segment per batch; per-segment 16-head attention (hd=128);
  fused qkv and out projections.  Output [2, 4096, 2048] f32.

Sharding: the 32 (batch, segment) instances are independent -> 4 per core.
Host pre-gathers the dilated tokens, pre-transposes/pre-tiles operands and
casts to bf16 (compute precision; measured end-to-end rel err ~5e-3).

The kernel is PE-bound (93%+ occupancy): 1.11M matmul columns/core at
2.4GHz is ~464us.  Beyond the baseline, this version
  - computes softmax denominators with a per-head batched GpSimd
    partition_all_reduce instead of ones-matmuls (-32k PE columns),
  - interleaves attention seg-pairs between the q- and k-projection
    chunks so the ACT exp latency always hides under ~7us of projection,
  - emits the output projection feature-major (outT[e, tok]) so the
    stationary operand is a Wout tile reused for 1024 moving columns
    (half the LDWEIGHTS of the token-major form); host transposes back,
  - streams xst by token-quarters and runs the first v-chunk
    quarter-major so the PE starts ~4us earlier during the cold 4MB
    xst delivery.

Per-core dataflow (all matmuls K=128, bf16):
  qkv proj   : feature-major  qkvT[e, tok] = W-tile.T @ xsT-tile  (accum 16)
  scores     : scoresT[lk, lq] = kT.T @ qT  (per seg, head)
  softmax    : exp on ScalarE (scale=1/sqrt(hd); scores provably in
               [-6, 6] so no max subtraction); denominators via DVE
               chunk-add + GpSimd partition_all_reduce; normalize on DVE
               at the AV psum drain.
  AV         : outT[hd, lq] = v[lk, hd].T @ expT[lk, lq]
  out proj   : outT[e, tok] = WoutT-tile.T @ aT-tile  (accum 16 heads)
b_out is applied on the host (purely linear post-op); b_qkv is applied
on-chip (ScalarE bias) since it feeds the softmax nonlinearity.
"""

import numpy as np
import ml_dtypes

B = 2
S = 8192
D = 2048
H = 16
HD = 128
SEGMENT = 512
DIL = 2
NSEG = S // SEGMENT          # 16
L = SEGMENT // DIL           # 256 dilated tokens per segment
N_CORES = 8
PAIRS = B * NSEG             # 32 independent (b, n) instances
SPC = PAIRS // N_CORES       # 4 segments per core
TOK = SPC * L                # 1024 tokens per core
DT = D // 128                # 16 contraction tiles
NCHUNK = 3 * D // 128        # 48 qkv feature chunks (16 q, 16 k, 16 v)
SCALE = 1.0 / float(np.sqrt(HD))

_PROGRAM = None


def _build_program():
    import concourse.bass as bass
    import concourse.bacc as bacc
    import concourse.tile as tile
    from concourse import mybir
    from concourse import bass_isa

    BF = mybir.dt.bfloat16
    F32 = mybir.dt.float32
    ts = bass.ts

    nc = bacc.Bacc("TRN2", target_bir_lowering=False, debug=False,
                   num_devices=N_CORES)

    xst_d = nc.dram_tensor("xst", [4, 128, DT, 256], BF, kind="ExternalInput")
    wqkv_d = nc.dram_tensor("wqkv_t", [NCHUNK, 128, DT * 128], BF,
                            kind="ExternalInput")
    wout_d = nc.dram_tensor("wout_t", [DT, 128, H * 128], BF,
                            kind="ExternalInput")
    bq_d = nc.dram_tensor("bq_t", [128, NCHUNK], F32, kind="ExternalInput")
    out_d = nc.dram_tensor("out", [D, TOK], F32, kind="ExternalOutput")

    with tile.TileContext(nc) as tc:
        with (
            tc.tile_pool(name="const", bufs=1) as const_p,
            tc.tile_pool(name="big", bufs=1) as big_p,
            tc.tile_pool(name="wq", bufs=8) as w_p,
            tc.tile_pool(name="qk", bufs=4) as qk_p,
            tc.tile_pool(name="vt", bufs=2) as vt_p,
            tc.tile_pool(name="ex", bufs=4) as ex_p,
            tc.tile_pool(name="st", bufs=2) as st_p,
            tc.tile_pool(name="ou", bufs=2) as ou_p,
            tc.tile_pool(name="pp", bufs=4, space="PSUM") as pp_p,
            tc.tile_pool(name="pa", bufs=2, space="PSUM") as pa_p,
        ):
            bq_sb = const_p.tile([128, NCHUNK], F32)
            nc.sync.dma_start(out=bq_sb[:], in_=bq_d[:])

            # Cold-start ordering: the PE's first work (v chunk 0, index
            # 32) needs the first quarter of its W chunk and the first
            # token-quarter of xst.  Both live in token-quarter-major
            # layouts so every piece is one fully-linear DMA, and the
            # first matmul can start after ~1.1MB instead of ~4.5MB.
            first_w = w_p.tile([128, DT * 128], BF, tag="w", name="first_w")
            nc.sync.dma_start(out=first_w[:, 0:512], in_=wqkv_d[32][:, 0:512])
            xst_sb = big_p.tile([128, 4, DT, 256], BF)
            nc.sync.dma_start(out=xst_sb[:, 0], in_=xst_d[0])
            for kk in range(1, 4):
                nc.sync.dma_start(out=first_w[:, ts(kk, 512)],
                                  in_=wqkv_d[32][:, ts(kk, 512)])
            for q in range(1, 4):
                nc.sync.dma_start(out=xst_sb[:, q], in_=xst_d[q])
            vtok_sb = big_p.tile([128, H, SPC * 2, 128], BF)
            # head-major so the out-projection's [seg-pair, head] reads have
            # tight per-head dependency ranges (head hh's matmuls wait only
            # on head hh's normalize, not on later heads)
            aT_sb = big_p.tile([128, H, SPC, L], BF)

            def proj_chunk(c, out_tile, wck=None, quarter_major=False):
                """qkvT chunk c: out_tile[128, TOK] bf16 = W-chunk.T @ xsT + b."""
                if wck is None:
                    wck = w_p.tile([128, DT * 128], BF, tag="w")
                    nc.sync.dma_start(out=wck[:], in_=wqkv_d[c])
                if quarter_major:
                    # first chunk while xst streams in: consume one
                    # token-quarter at a time so matmuls start on quarter 0
                    for q in range(4):
                        psq = pp_p.tile([128, 512], F32, tag="pp", name="psq")
                        for dt in range(DT):
                            nc.tensor.matmul(
                                psq[:, 0:256],
                                wck[:, ts(dt, 128)],
                                xst_sb[:, q, dt, :],
                                start=(dt == 0),
                                stop=(dt == DT - 1),
                            )
                        if q % 2 == 0:
                            nc.scalar.activation(
                                out=out_tile[:, ts(q, 256)],
                                in_=psq[:, 0:256],
                                func=mybir.ActivationFunctionType.Identity,
                                bias=bq_sb[:, c:c + 1],
                                scale=1.0,
                            )
                        else:
                            nc.vector.tensor_scalar_add(
                                out=out_tile[:, ts(q, 256)],
                                in0=psq[:, 0:256],
                                scalar1=bq_sb[:, c:c + 1],
                            )
                    return
                pss = [pp_p.tile([128, 512], F32, tag="pp", name=f"ps{half}")
                       for half in range(2)]
                for dt in range(DT):
                    for half in range(2):
                        nc.tensor.matmul(
                            pss[half][:],
                            wck[:, ts(dt, 128)],
                            xst_sb[:, 2 * half:2 * half + 2, dt, :],
                            start=(dt == 0),
                            stop=(dt == DT - 1),
                        )
                # drain one half on ScalarE, the other on DVE: halves the
                # psum-free latency and decongests ScalarE (which also runs
                # the attention exps)
                nc.scalar.activation(
                    out=out_tile[:, ts(0, 512)],
                    in_=pss[0][:],
                    func=mybir.ActivationFunctionType.Identity,
                    bias=bq_sb[:, c:c + 1],
                    scale=1.0,
                )
                nc.vector.tensor_scalar_add(
                    out=out_tile[:, ts(1, 512)],
                    in0=pss[1][:],
                    scalar1=bq_sb[:, c:c + 1],
                )

            # ---- v projection (feature-major) + transpose to token-major ----
            # One transposing DMA per head (xbar transpose, ~261GB/s): row
            # tok = tc*128+p of vt.T lands at vtok[p, tc, :], exactly the AV
            # stationary layout.  Emitted one chunk behind the projection so
            # the DMA never waits on the ScalarE psum->sbuf drain.
            def v_transposes(h, vt_tile):
                nc.sync.dma_start(out=vtok_sb[:, h, :, :], in_=vt_tile[:],
                                  transpose=True)

            prev_v = None
            for h in range(H):
                vt_tile = vt_p.tile([128, TOK], BF, tag="vt")
                proj_chunk(32 + h, vt_tile,
                           wck=first_w if h == 0 else None,
                           quarter_major=(h == 0))
                if prev_v is not None:
                    v_transposes(h - 1, prev_v)
                prev_v = vt_tile
            v_transposes(H - 1, prev_v)

            # ---- per-head attention, seg-pair interleaved with projection ----
            # scoresT[lk, lq] directly (operands swapped): exp is
            # layout-agnostic (scores provably small -> no max pass), expT
            # feeds AV untransposed.  Softmax denominators: DVE adds the two
            # lk chunks of expT, one batched GpSimd partition_all_reduce per
            # head sums over partitions (all partitions get the result), DVE
            # reciprocal + multiply normalize at the AV psum drain.
            # Schedule: the ~0.7us ACT exp of a seg-pair always has a full
            # projection chunk (~7us) between its scoresT and its AV, so the
            # PE stream never waits on ScalarE.
            head_state = {}

            def emit_scores_pair(h, qh, kh, pair):
                """scoresT + exp + chunk-add + denominator all-reduce for
                segs 2*pair, 2*pair+1.  The GpSimd all-reduce is emitted
                here (per pair, ~3.7us) so it runs during the next
                projection chunk, well before the pair's normalize."""
                st = head_state[h]
                es = ex_p.tile([128, 2, L], BF, tag="es", bufs=4, name="es")
                for i, seg in enumerate((2 * pair, 2 * pair + 1)):
                    scT = pa_p.tile([128, 2, L], F32, tag="pa", name="scT")
                    for lkc in range(2):
                        nc.tensor.matmul(
                            scT[:, lkc, :],
                            kh[:, seg * L + lkc * 128: seg * L + (lkc + 1) * 128],
                            qh[:, seg * L:(seg + 1) * L],
                        )
                    e_t = ex_p.tile([128, 2, L], BF, tag="ex")
                    nc.scalar.activation(
                        out=e_t[:],
                        in_=scT[:],
                        func=mybir.ActivationFunctionType.Exp,
                        scale=SCALE,
                    )
                    st["e"][seg] = e_t
                    nc.vector.tensor_add(es[:, i, :], e_t[:, 0, :], e_t[:, 1, :])
                den = st_p.tile([128, 2, L], F32, tag="den", bufs=2, name="den")
                nc.gpsimd.partition_all_reduce(
                    den[:], es[:], 128, bass_isa.ReduceOp.add)
                st["den"][pair] = den

            def emit_av_pair(h, pair):
                """AV for segs 2*pair, 2*pair+1 into one psum bank."""
                st = head_state[h]
                avs = pa_p.tile([128, 2, L], F32, tag="pav", bufs=2,
                                name="avs")
                st["av"][pair] = avs
                for i, seg in enumerate((2 * pair, 2 * pair + 1)):
                    e_t = st["e"][seg]
                    for lkc in range(2):
                        nc.tensor.matmul(
                            avs[:, i, :],
                            vtok_sb[:, h, seg * 2 + lkc, :],
                            e_t[:, lkc, :],
                            start=(lkc == 0),
                            stop=(lkc == 1),
                        )

            def emit_norm_pair(h, pair):
                """Normalize the pair's AV by its denominators -> aT_sb."""
                st = head_state[h]
                inv = st_p.tile([128, 2, L], F32, tag="inv", bufs=2, name="inv")
                nc.vector.reciprocal_approx_fast(out=inv[:], in_=st["den"][pair][:])
                avs = st["av"][pair]
                for i, seg in enumerate((2 * pair, 2 * pair + 1)):
                    nc.vector.tensor_mul(
                        aT_sb[:, h, seg, :], avs[:, i, :], inv[:, i, :])

            def start_head(h, qh, kh):
                head_state[h] = {
                    "q": qh, "k": kh,
                    "e": [None] * SPC,
                    "den": [None] * 2,
                    "av": [None] * 2,
                }

            # Prefetch the first two Wout e-chunk blocks now: their
            # dma_starts land early in the Sync stream, so the transfers
            # run during the attention phase instead of stalling the
            # out-projection start by ~3us.
            wo_pre = []
            for ec in range(2):
                wo_ec = w_p.tile([128, H, 128], BF, tag="wo", bufs=2,
                                 name="wo_ec")
                nc.sync.dma_start(out=wo_ec[:], in_=wout_d[ec])
                wo_pre.append(wo_ec)

            prev = None
            for h in range(H):
                qh = qk_p.tile([128, TOK], BF, tag="qk")
                kh = qk_p.tile([128, TOK], BF, tag="qk")
                proj_chunk(h, qh)
                if prev is not None:
                    # block X: AV + normalize (prev, pair 0), scoresT(prev,
                    # pair 1)
                    emit_av_pair(prev, 0)
                    emit_norm_pair(prev, 0)
                    emit_scores_pair(prev, head_state[prev]["q"],
                                     head_state[prev]["k"], 1)
                proj_chunk(16 + h, kh)
                if prev is not None:
                    # block Y: AV + normalize (prev, pair 1), then
                    # scoresT(h, pair 0)
                    emit_av_pair(prev, 1)
                    emit_norm_pair(prev, 1)
                    del head_state[prev]
                start_head(h, qh, kh)
                emit_scores_pair(h, qh, kh, 0)
                prev = h
            # drain last head
            emit_av_pair(prev, 0)
            emit_norm_pair(prev, 0)
            emit_scores_pair(prev, head_state[prev]["q"],
                             head_state[prev]["k"], 1)
            emit_av_pair(prev, 1)
            emit_norm_pair(prev, 1)
            del head_state[prev]

            # ---- output projection (feature-major: outT[e, tok]) ----
            # Stationary = Wout tile (one per (e-chunk, head), reused for
            # 1024 moving columns -> 256 LDWEIGHTS total).  Wout streams in
            # sixteen 512KB e-chunk blocks (linear DMAs, 2 prefetched).
            def op_mms(wo_ec, pos, hh):
                for sp in range(2):
                    nc.tensor.matmul(
                        pos[sp][:],
                        wo_ec[:, hh, :],
                        aT_sb[:, hh, 2 * sp:2 * sp + 2, :],
                        start=(hh == 0),
                        stop=(hh == H - 1),
                    )

            def op_drain(ec, pos, last=False):
                for sp in range(2):
                    if last and sp == 1:
                        # split the last drain so the final store DMA
                        # starts half a tile earlier
                        for qq in range(2):
                            ob = ou_p.tile([128, 256], F32, tag="ou2")
                            nc.vector.tensor_copy(
                                out=ob[:], in_=pos[sp][:, ts(qq, 256)])
                            nc.sync.dma_start(
                                out=out_d[ec * 128:(ec + 1) * 128,
                                          sp * 512 + qq * 256:
                                          sp * 512 + (qq + 1) * 256],
                                in_=ob[:],
                            )
                    else:
                        ob = ou_p.tile([128, 512], F32, tag="ou")
                        nc.vector.tensor_copy(out=ob[:], in_=pos[sp][:])
                        nc.sync.dma_start(
                            out=out_d[ec * 128:(ec + 1) * 128, ts(sp, 512)],
                            in_=ob[:],
                        )

            # Blocks 0 and 1 run interleaved with their head-15 terms
            # deferred: ~13us of head-0..14 accumulation covers the last
            # head's exp -> all-reduce -> normalize drain, so the PE never
            # waits on it.
            pos01 = [[pp_p.tile([128, 512], F32, tag="pp", name=f"po{ec}{sp}")
                      for sp in range(2)] for ec in range(2)]
            for hh in range(H - 1):
                op_mms(wo_pre[0], pos01[0], hh)
                op_mms(wo_pre[1], pos01[1], hh)
            op_mms(wo_pre[0], pos01[0], H - 1)
            op_drain(0, pos01[0])
            op_mms(wo_pre[1], pos01[1], H - 1)
            op_drain(1, pos01[1])

            for ec in range(2, DT):
                wo_ec = w_p.tile([128, H, 128], BF, tag="wo", bufs=2,
                                 name="wo_ec")
                nc.sync.dma_start(out=wo_ec[:], in_=wout_d[ec])
                pos = [pp_p.tile([128, 512], F32, tag="pp", name=f"po{sp}")
                       for sp in range(2)]
                for hh in range(H):
                    op_mms(wo_ec, pos, hh)
                op_drain(ec, pos, last=(ec == DT - 1))

    nc.compile()
    _dedupe_ldweights(nc)
    return nc


def _dedupe_ldweights(nc):
    """Drop InstLdweights whose weights are already resident in the PE array.

    tile_legalize emits one LDWEIGHTS per matmul; consecutive matmuls that
    share the stationary operand (projection token-halves, out-proj seg
    pairs) reload identical weights, costing ~97ns of PE pipe each.  Walk
    each block's PE stream tracking the loaded-weights key and delete
    reloads.  Only semaphore-free LDWEIGHTS are dropped, so the sync graph
    is untouched; EVENT_SEMAPHORE/DRAIN between pairs don't disturb the
    array, any other PE instruction conservatively invalidates the key.
    """
    from concourse import mybir

    PE = mybir.EngineType.PE
    dropped = 0
    for f in nc.m.functions:
        for blk in f.blocks:
            insts = blk.instructions
            loaded = None
            to_drop = []
            for idx, x in enumerate(insts):
                if getattr(x, "engine", None) != PE:
                    continue
                nm = type(x).__name__
                if nm == "InstLdweights":
                    si = x.sync_info
                    clean = si is None or (not si.on_wait and not si.on_update)
                    key = (str(x.ins[0]), str(x.is_transpose),
                           str(x.perf_mode), str(x.tile_position))
                    if clean and loaded == key:
                        to_drop.append(idx)
                    else:
                        loaded = key
                elif nm == "InstMatmult":
                    continue
                elif nm in ("InstEventSemaphore", "InstDrain"):
                    continue
                else:
                    loaded = None
            for idx in reversed(to_drop):
                del insts[idx]
            blk.instructions = insts
            dropped += len(to_drop)
    return dropped


def get_program():
    global _PROGRAM
    if _PROGRAM is None:
        _PROGRAM = _build_program()
    return _PROGRAM


def make_in_maps(x, Wqkv, b_qkv):
    """Host-side shard + layout prep (bf16 casts, transposes, tiling)."""
    bf16 = ml_dtypes.bfloat16
    x = np.asarray(x, dtype=np.float32)
    Wqkv = np.asarray(Wqkv, dtype=np.float32)
    b_qkv = np.asarray(b_qkv, dtype=np.float32)

    xs = x.reshape(B, NSEG, SEGMENT, D)[:, :, ::DIL, :]     # [2,16,256,2048]
    xs_flat = xs.reshape(PAIRS, L, D)

    # lhsT tiles packed partition-major: wt[c, p, dt*128+j] = WqkvT[dt*128+p,
    # c*128+j] so one chunk is a single linear per-partition DMA.
    wt = np.ascontiguousarray(
        Wqkv.reshape(NCHUNK, 128, DT, 128).transpose(0, 3, 2, 1)
        .reshape(NCHUNK, 128, DT * 128)
    ).astype(bf16)                                          # [48,128,2048]
    bqt = np.ascontiguousarray(b_qkv.reshape(NCHUNK, 128).T)  # [128,48] f32

    in_maps = []
    for i in range(N_CORES):
        tok = xs_flat[SPC * i:SPC * (i + 1)].reshape(TOK, D)
        # token-quarter-major: xst[q, p, dt, j] = xsT[dt*128+p, q*256+j]
        # so each quarter is one fully-linear DMA.
        xst = np.ascontiguousarray(
            tok.T.reshape(DT, 128, 4, 256).transpose(2, 1, 0, 3)).astype(bf16)
        in_maps.append({"xst": xst, "wqkv_t": wt, "bq_t": bqt})
    return in_maps


def make_wout_tiled(Wout):
    Wout = np.asarray(Wout, dtype=np.float32)
    # wout_t[ec, p, h*128+j] = Wout[ec*128+j, h*128+p]: per e-chunk block
    # of per-head lhsT tiles, one linear 512KB DMA each.
    return np.ascontiguousarray(
        Wout.reshape(DT, 128, H, 128).transpose(0, 3, 2, 1)
        .reshape(DT, 128, H * 128)).astype(ml_dtypes.bfloat16)


def kernel(x, Wqkv, b_qkv, Wout, b_out):
    from concourse import bass_utils

    nc = get_program()
    in_maps = make_in_maps(x, Wqkv, b_qkv)
    wot = make_wout_tiled(Wout)
    for m in in_maps:
        m["wout_t"] = wot

    res = bass_utils.run_bass_kernel_spmd(
        nc, in_maps, core_ids=list(range(N_CORES)))
    # out is feature-major [D, TOK] per core -> transpose back to [TOK, D]
    outs = [np.ascontiguousarray(res.results[i]["out"].T)
            for i in range(N_CORES)]
    full = np.concatenate(outs, axis=0) + np.asarray(b_out, dtype=np.float32)
    return np.ascontiguousarray(full.reshape(B, NSEG * L, D), dtype=np.float32)
